# revision 8
# baseline (speedup 1.0000x reference)
"""Bidirectional 2-layer GRU + FC kernel for Trainium2 (8 NeuronCores).

Strategy:
  - Cores 0-3 run layer-0 FORWARD for batch slice [8p, 8p+8); cores 4-7 run
    layer-0 BACKWARD for the same slices (fed time-reversed x + backward
    weights via per-core in_maps; the device program is SPMD-uniform).
  - gx (input projections + biases) are precomputed with big fp32r matmuls.
  - The recurrence runs in transposed layout: gate rows on partitions,
    batch on the free dim.  Per step: 48 fp32r matmuls (w_hh.T stationary
    chunks) + K=1 bias matmuls, then sigmoid/tanh on ScalarE and
    elementwise on VectorE.
  - A pairwise AllGather {i, i+4} shares the layer-0 trajectories; both pair
    members then redundantly compute layer-1 forward for their 8 batches
    (recurrence wall-time is batch-independent, so redundancy is free).
  - Layer-1 backward contributes only its t=T-1 state to the output, which
    takes a single step from h0=0.  FC bias is fused into an ACT Identity.
"""

import contextlib

import numpy as np

B, T_FULL, I_IN, H, C = 32, 512, 256, 512, 10
NCORES = 8
BA = 8          # batch per core
MCH = 12        # 3H / 128 gate-row chunks
KH = 4          # H / 128 contraction chunks

_PROGRAM_CACHE = {}


def _build(T):
    import concourse.bacc as bacc
    import concourse.mybir as mybir
    import concourse.tile as tile

    f32 = mybir.dt.float32
    f32r = mybir.dt.float32r
    SIG = mybir.ActivationFunctionType.Sigmoid
    TANH = mybir.ActivationFunctionType.Tanh
    IDENT = mybir.ActivationFunctionType.Identity
    ALU = mybir.AluOpType

    TB = min(64, T)          # timestep block for the big matmul phases
    NTB = T // TB

    nc = bacc.Bacc("TRN2", target_bir_lowering=False, debug=False,
                   num_devices=NCORES)

    def inp(name, shape, dt=f32r):
        return nc.dram_tensor(name, shape, dt, kind="ExternalInput").ap()

    xT = inp("xT", [I_IN, T, BA])               # own batch slice, own time order
    wihT0 = inp("wihT0", [I_IN, 3 * H])         # own direction w_ih.T
    bias0 = inp("bias0", [1, 3 * H])            # b_ih + b_hh (rz); n part = b_ih_n
    bhn0 = inp("bhn0", [1, H])                  # b_hh n part
    whhT0 = inp("whhT0", [H, 3 * H])
    wih1T_f = inp("wih1T_f", [H, 3 * H])        # w_ih_l1f.T rows 0:H   (f0 input)
    wih1T_b = inp("wih1T_b", [H, 3 * H])        # w_ih_l1f.T rows H:2H  (b0 input)
    bias1 = inp("bias1", [1, 3 * H])
    bhn1 = inp("bhn1", [1, H])
    whh1T = inp("whh1T", [H, 3 * H])
    wih1bT = inp("wih1bT", [2 * H, 3 * H])      # w_ih_l1b.T
    bias1b_sc = inp("bias1b_sc", [128, MCH], f32)   # per m-chunk column
    bhn1b_sc = inp("bhn1b_sc", [128, KH], f32)
    fcwT = inp("fcwT", [2 * H, C])
    fcb = inp("fcb", [C, 1], f32)

    outT = nc.dram_tensor("outT", [C, BA], f32, kind="ExternalOutput").ap()

    with tile.TileContext(nc) as tc, contextlib.ExitStack() as ctx:
        # ---------------- DRAM scratch (Tile-tracked) ----------------
        dramp = ctx.enter_context(tc.tile_pool(name="dramp", bufs=1, space="DRAM"))
        gx0 = dramp.tile([T, 128, MCH * BA], f32, tag="gx0")
        hbuf = dramp.tile([T, 128, KH * BA], f32r, tag="hbuf")
        agbuf = dramp.tile([2, T, 128, KH * BA], f32r, tag="agbuf")
        gx1f = dramp.tile([T, 128, MCH * BA], f32, tag="gx1f")
        gx1b = dramp.tile([T, 128, MCH * BA], f32, tag="gx1b")

        # ---------------- persistent SBUF (one pool, distinct tags) --------
        constp = ctx.enter_context(tc.tile_pool(name="constp", bufs=1))

        def const_tile(shape, dt, tag):
            return constp.tile(shape, dt, tag=tag, name=tag)

        whhT0_sb = const_tile([128, KH, 3 * H], f32r, "whhT0_sb")
        nc.sync.dma_start(whhT0_sb[:], whhT0.rearrange("(k p) m -> p k m", p=128))
        bias0_sb = const_tile([128, 3 * H], f32r, "bias0_sb")[0:1, :]
        nc.sync.dma_start(bias0_sb, bias0[:])
        bhn0_sb = const_tile([128, H], f32r, "bhn0_sb")[0:1, :]
        nc.sync.dma_start(bhn0_sb, bhn0[:])
        ones_big = const_tile([128, TB * BA], f32, "ones_big")[0:1, :]
        nc.vector.memset(ones_big, 1.0)
        ones_ba = ones_big[:, 0:BA]

        whh1_sb = const_tile([128, KH, 3 * H], f32r, "whh1_sb")
        nc.sync.dma_start(whh1_sb[:], whh1T.rearrange("(k p) m -> p k m", p=128))
        bhn1_sb = const_tile([128, H], f32r, "bhn1_sb")[0:1, :]
        nc.sync.dma_start(bhn1_sb, bhn1[:])
        bias1_sb = const_tile([128, 3 * H], f32r, "bias1_sb")[0:1, :]
        nc.sync.dma_start(bias1_sb, bias1[:])

        b1b_sb = const_tile([128, MCH], f32, "b1b_sb")
        nc.sync.dma_start(b1b_sb[:], bias1b_sc[:])
        bhn1b_sb = const_tile([128, KH], f32, "bhn1b_sb")
        nc.sync.dma_start(bhn1b_sb[:], bhn1b_sc[:])
        fcw_sb = const_tile([128, 2 * KH, C], f32r, "fcw_sb")
        nc.sync.dma_start(fcw_sb[:], fcwT.rearrange("(k p) c -> p k c", p=128))
        fcb_sb = const_tile([128, 1], f32, "fcb_sb")[0:C, :]
        nc.sync.dma_start(fcb_sb, fcb[:])

        # h-state tiles for both scans + l1b results (long-lived)
        h0a = const_tile([128, KH * BA], f32r, "h0a")
        h0b = const_tile([128, KH * BA], f32r, "h0b")
        h1a = const_tile([128, KH * BA], f32r, "h1a")
        h1b_ = const_tile([128, KH * BA], f32r, "h1b_")
        zscr = const_tile([128, KH * BA], f32, "zscr")
        nc.vector.memset(zscr[:], 0.0)
        gxl = const_tile([128, MCH * BA], f32, "gxl")
        rl = const_tile([128, 4 * BA], f32, "rl")
        zpl = const_tile([128, 4 * BA], f32, "zpl")
        n1l = const_tile([128, 4 * BA], f32, "n1l")
        ntl = const_tile([128, 4 * BA], f32, "ntl")
        h1bk = const_tile([128, 4 * BA], f32r, "h1bk")
        x1last = const_tile([128, 2 * KH, BA], f32r, "x1last")
        out_sb = const_tile([128, BA], f32, "out_sb")[0:C, :]

        # ================ Phase 0: gx0 = w_ih0 @ x.T + bias0 ================
        with contextlib.ExitStack() as p0s:
            wp0 = p0s.enter_context(tc.tile_pool(name="wp0", bufs=1))
            wihT0_sb = wp0.tile([128, 2, 3 * H], f32r, tag="wihT0_sb")
            nc.sync.dma_start(wihT0_sb[:],
                              wihT0.rearrange("(k p) m -> p k m", p=128))
            p0_mv = p0s.enter_context(tc.tile_pool(name="p0_mv", bufs=3))
            p0_ps = p0s.enter_context(tc.tile_pool(name="p0_ps", bufs=4,
                                                   space="PSUM"))
            p0_out = p0s.enter_context(tc.tile_pool(name="p0_out", bufs=4))

            xT_r = xT.rearrange("(k p) t b -> p k t b", p=128)
            for c in range(NTB):
                mv = p0_mv.tile([128, 2, TB, BA], f32r, tag="mv")
                nc.sync.dma_start(mv[:], xT_r[:, :, c * TB:(c + 1) * TB, :])
                for m in range(MCH):
                    ps = p0_ps.tile([128, TB * BA], f32, tag="ps")
                    for k in range(2):
                        nc.tensor.matmul(ps[:],
                                         wihT0_sb[:, k, 128 * m:128 * (m + 1)],
                                         mv[:, k, :, :],
                                         start=(k == 0), stop=False)
                    nc.tensor.matmul(ps[:], bias0_sb[:, 128 * m:128 * (m + 1)],
                                     ones_big.bitcast(f32r),
                                     start=False, stop=True)
                    ot = p0_out.tile([128, TB, BA], f32, tag="ot")
                    flat = ot[:].rearrange("p t b -> p (t b)")
                    if m % 2 == 0:
                        nc.vector.tensor_copy(flat, ps[:])
                    else:
                        nc.scalar.copy(flat, ps[:])
                    nc.scalar.dma_start(
                        gx0[c * TB:(c + 1) * TB, :, BA * m:BA * (m + 1)]
                        .transpose([1, 0, 2]), ot[:])

        # ================ generic GRU scan ================
        def scan(h_even, h_odd, gx_load, whh_sb, bhn_sb, store_h, name):
            h_t = [h_even, h_odd]
            nc.vector.tensor_copy(h_t[0][:], zscr[:])
            with contextlib.ExitStack() as ss:
                ps_rz_p = ss.enter_context(
                    tc.tile_pool(name=f"{name}_psrz", bufs=2, space="PSUM"))
                ps_n_p = ss.enter_context(
                    tc.tile_pool(name=f"{name}_psn", bufs=2, space="PSUM"))
                gxp = ss.enter_context(tc.tile_pool(name=f"{name}_gx", bufs=4))
                gp = ss.enter_context(tc.tile_pool(name=f"{name}_g", bufs=3))

                for t in range(T):
                    h_cur, h_nxt = h_t[t % 2], h_t[(t + 1) % 2]
                    gx = gx_load(t, gxp)
                    ps_rz = ps_rz_p.tile([128, 8 * BA], f32, tag="psrz")
                    ps_n = ps_n_p.tile([128, 4 * BA], f32, tag="psn")
                    hr = h_cur[:]
                    for m in range(8):
                        for k in range(KH):
                            nc.tensor.matmul(ps_rz[:, BA * m:BA * (m + 1)],
                                             whh_sb[:, k, 128 * m:128 * (m + 1)],
                                             hr[:, BA * k:BA * (k + 1)],
                                             start=(k == 0), stop=(k == KH - 1))
                    for m in range(8, MCH):
                        mm = m - 8
                        for k in range(KH):
                            nc.tensor.matmul(ps_n[:, BA * mm:BA * (mm + 1)],
                                             whh_sb[:, k, 128 * m:128 * (m + 1)],
                                             hr[:, BA * k:BA * (k + 1)],
                                             start=(k == 0), stop=False)
                        nc.tensor.matmul(ps_n[:, BA * mm:BA * (mm + 1)],
                                         bhn_sb[:, 128 * mm:128 * (mm + 1)],
                                         ones_ba.bitcast(f32r),
                                         start=False, stop=True)
                    rz_pre = gp.tile([128, 8 * BA], f32, tag="rz_pre")
                    nc.vector.tensor_add(rz_pre[:], ps_rz[:], gx[:, 0:8 * BA])
                    rz = gp.tile([128, 8 * BA], f32, tag="rz")
                    nc.scalar.activation(rz[:], rz_pre[:], SIG)
                    n1 = gp.tile([128, 4 * BA], f32, tag="n1")
                    nc.vector.tensor_mul(n1[:], ps_n[:], rz[:, 0:4 * BA])
                    n2 = gp.tile([128, 4 * BA], f32, tag="n2")
                    nc.vector.tensor_add(n2[:], n1[:], gx[:, 8 * BA:MCH * BA])
                    nt = gp.tile([128, 4 * BA], f32, tag="nt")
                    nc.scalar.activation(nt[:], n2[:], TANH)
                    d = gp.tile([128, 4 * BA], f32, tag="d")
                    nc.vector.tensor_sub(d[:], h_cur[:].bitcast(f32), nt[:])
                    e = gp.tile([128, 4 * BA], f32, tag="e")
                    nc.vector.tensor_mul(e[:], d[:], rz[:, 4 * BA:8 * BA])
                    nc.vector.tensor_add(h_nxt[:], e[:], nt[:])
                    if store_h is not None:
                        nc.scalar.dma_start(store_h[t], h_nxt[:])
            return h_t[T % 2]

        # ================ Phase A: layer-0 scan (own direction) ============
        def gx0_load(t, pool):
            g = pool.tile([128, MCH * BA], f32, tag="gx")
            nc.sync.dma_start(g[:], gx0[t])
            return g

        scan(h0a, h0b, gx0_load, whhT0_sb, bhn0_sb, hbuf, "s0")

        # ================ AllGather pair {i, i+4} ================
        nc.gpsimd.collective_compute(
            "AllGather", ALU.bypass,
            replica_groups=[[0, 4], [1, 5], [2, 6], [3, 7]],
            ins=[hbuf[:]], outs=[agbuf[:]])

        # ================ Phase gx1: two passes over agbuf ================
        with contextlib.ExitStack() as g1s:
            w1p = g1s.enter_context(tc.tile_pool(name="w1p", bufs=2))
            g1_mv = g1s.enter_context(tc.tile_pool(name="g1_mv", bufs=3))
            g1_ps = g1s.enter_context(tc.tile_pool(name="g1_ps", bufs=4,
                                                   space="PSUM"))
            g1_out = g1s.enter_context(tc.tile_pool(name="g1_out", bufs=4))
            for pi, (wsrc, dst, with_bias) in enumerate(
                    [(wih1T_f, gx1f, True), (wih1T_b, gx1b, False)]):
                w_sb = w1p.tile([128, KH, 3 * H], f32r, tag="w1")
                nc.sync.dma_start(w_sb[:],
                                  wsrc.rearrange("(k p) m -> p k m", p=128))
                for c in range(NTB):
                    mv = g1_mv.tile([128, KH, TB, BA], f32r, tag="g1mv")
                    for j in range(KH):
                        nc.sync.dma_start(
                            mv[:, j, :, :],
                            agbuf[pi, c * TB:(c + 1) * TB, :,
                                  BA * j:BA * (j + 1)].transpose([1, 0, 2]))
                    for m in range(MCH):
                        ps = g1_ps.tile([128, TB * BA], f32, tag="ps")
                        for k in range(KH):
                            nc.tensor.matmul(
                                ps[:], w_sb[:, k, 128 * m:128 * (m + 1)],
                                mv[:, k, :, :],
                                start=(k == 0),
                                stop=(not with_bias and k == KH - 1))
                        if with_bias:
                            nc.tensor.matmul(
                                ps[:], bias1_sb[:, 128 * m:128 * (m + 1)],
                                ones_big.bitcast(f32r), start=False, stop=True)
                        ot = g1_out.tile([128, TB, BA], f32, tag="ot")
                        flat = ot[:].rearrange("p t b -> p (t b)")
                        if m % 2 == 0:
                            nc.vector.tensor_copy(flat, ps[:])
                        else:
                            nc.scalar.copy(flat, ps[:])
                        nc.scalar.dma_start(
                            dst[c * TB:(c + 1) * TB, :, BA * m:BA * (m + 1)]
                            .transpose([1, 0, 2]), ot[:])

        # ================ layer-1 backward: single step at t = T-1 =========
        with contextlib.ExitStack() as lbs:
            lb_wp = lbs.enter_context(tc.tile_pool(name="lb_wp", bufs=1))
            l1b_w = lb_wp.tile([128, 2 * KH, 3 * H], f32r, tag="l1b_w")
            nc.sync.dma_start(l1b_w[:],
                              wih1bT.rearrange("(k p) m -> p k m", p=128))
            for j in range(KH):
                nc.sync.dma_start(x1last[:, j, :],
                                  agbuf[0, T - 1, :, BA * j:BA * (j + 1)])
                nc.sync.dma_start(x1last[:, KH + j, :],
                                  agbuf[1, 0, :, BA * j:BA * (j + 1)])
            l1b_ps_p = lbs.enter_context(
                tc.tile_pool(name="l1b_ps", bufs=1, space="PSUM"))
            l1b_ps = l1b_ps_p.tile([128, MCH * BA], f32, tag="l1b_ps")
            for m in range(MCH):
                for k in range(2 * KH):
                    nc.tensor.matmul(l1b_ps[:, BA * m:BA * (m + 1)],
                                     l1b_w[:, k, 128 * m:128 * (m + 1)],
                                     x1last[:, k, :],
                                     start=(k == 0), stop=(k == 2 * KH - 1))
            for m in range(MCH):
                nc.vector.tensor_scalar_add(gxl[:, BA * m:BA * (m + 1)],
                                            l1b_ps[:, BA * m:BA * (m + 1)],
                                            b1b_sb[:, m:m + 1])
            nc.scalar.activation(rl[:], gxl[:, 0:4 * BA], SIG)
            nc.scalar.activation(zpl[:], gxl[:, 4 * BA:8 * BA], SIG, scale=-1.0)
            for jj in range(KH):
                nc.vector.scalar_tensor_tensor(
                    n1l[:, BA * jj:BA * (jj + 1)], rl[:, BA * jj:BA * (jj + 1)],
                    bhn1b_sb[:, jj:jj + 1],
                    gxl[:, 8 * BA + BA * jj:8 * BA + BA * (jj + 1)],
                    ALU.mult, ALU.add)
            nc.scalar.activation(ntl[:], n1l[:], TANH)
            nc.vector.tensor_mul(h1bk[:], zpl[:], ntl[:])

        # ================ Phase B: layer-1 forward scan ================
        def gx1_load(t, pool):
            ga = pool.tile([128, MCH * BA], f32, tag="gxa")
            nc.sync.dma_start(ga[:], gx1f[t])
            gb = pool.tile([128, MCH * BA], f32, tag="gxb")
            nc.sync.dma_start(gb[:], gx1b[T - 1 - t])
            gs = pool.tile([128, MCH * BA], f32, tag="gxs")
            nc.vector.tensor_add(gs[:], ga[:], gb[:])
            return gs

        h1f = scan(h1a, h1b_, gx1_load, whh1_sb, bhn1_sb, None, "s1")

        # ================ FC ================
        with contextlib.ExitStack() as fcs:
            fc_ps_p = fcs.enter_context(
                tc.tile_pool(name="fc_ps", bufs=1, space="PSUM"))
            fc_ps_t = fc_ps_p.tile([128, BA], f32, tag="fc_ps", name="fc_ps")
            fc_ps = fc_ps_t[0:C, :]
            for k in range(KH):
                nc.tensor.matmul(fc_ps, fcw_sb[:, k, :],
                                 h1f[:, BA * k:BA * (k + 1)],
                                 start=(k == 0), stop=False)
            for k in range(KH):
                nc.tensor.matmul(fc_ps, fcw_sb[:, KH + k, :],
                                 h1bk[:, BA * k:BA * (k + 1)],
                                 start=False, stop=(k == KH - 1))
            nc.scalar.activation(out_sb, fc_ps, IDENT, bias=fcb_sb)
            nc.sync.dma_start(outT[:], out_sb)

    nc.compile()
    return nc


def _make_in_maps(inputs, T):
    x = np.asarray(inputs["x"], dtype=np.float32)

    def layer_params(wih, whh, bih, bhh):
        wih, whh = np.asarray(wih), np.asarray(whh)
        bih, bhh = np.asarray(bih), np.asarray(bhh)
        bias = (bih + bhh).astype(np.float32).copy()
        bias[2 * H:] = bih[2 * H:]
        return {
            "wihT": np.ascontiguousarray(wih.T, dtype=np.float32),
            "whhT": np.ascontiguousarray(whh.T, dtype=np.float32),
            "bias": bias.reshape(1, 3 * H),
            "bhn": bhh[2 * H:].reshape(1, H).astype(np.float32),
        }

    l0f = layer_params(inputs["w_ih_l0f"], inputs["w_hh_l0f"],
                       inputs["b_ih_l0f"], inputs["b_hh_l0f"])
    l0b = layer_params(inputs["w_ih_l0b"], inputs["w_hh_l0b"],
                       inputs["b_ih_l0b"], inputs["b_hh_l0b"])
    l1f = layer_params(inputs["w_ih_l1f"], inputs["w_hh_l1f"],
                       inputs["b_ih_l1f"], inputs["b_hh_l1f"])

    wih1fT = np.ascontiguousarray(np.asarray(inputs["w_ih_l1f"]).T,
                                  dtype=np.float32)  # [2H, 3H]
    wih1bT = np.ascontiguousarray(np.asarray(inputs["w_ih_l1b"]).T,
                                  dtype=np.float32)

    b1b = (np.asarray(inputs["b_ih_l1b"]) + np.asarray(inputs["b_hh_l1b"])
           ).astype(np.float32).copy()
    b1b[2 * H:] = np.asarray(inputs["b_ih_l1b"])[2 * H:]
    bias1b_sc = np.ascontiguousarray(b1b.reshape(MCH, 128).T)
    bhn1b_sc = np.ascontiguousarray(
        np.asarray(inputs["b_hh_l1b"])[2 * H:].reshape(KH, 128).T
        .astype(np.float32))

    fcwT = np.ascontiguousarray(np.asarray(inputs["fc_w"]).T, dtype=np.float32)
    fcb = np.asarray(inputs["fc_b"]).reshape(C, 1).astype(np.float32)

    common = {
        "wih1T_f": np.ascontiguousarray(wih1fT[:H]),
        "wih1T_b": np.ascontiguousarray(wih1fT[H:]),
        "bias1": l1f["bias"],
        "bhn1": l1f["bhn"],
        "whh1T": l1f["whhT"],
        "wih1bT": wih1bT,
        "bias1b_sc": bias1b_sc,
        "bhn1b_sc": bhn1b_sc,
        "fcwT": fcwT,
        "fcb": fcb,
    }

    in_maps = []
    for i in range(NCORES):
        p = i % 4
        back = i >= 4
        xs = x[8 * p:8 * p + 8, :T, :]
        if back:
            xs = xs[:, ::-1, :]
        xTl = np.ascontiguousarray(xs.transpose(2, 1, 0))  # [I, T, BA]
        lp = l0b if back else l0f
        m = {
            "xT": xTl,
            "wihT0": lp["wihT"],
            "bias0": lp["bias"],
            "bhn0": lp["bhn"],
            "whhT0": lp["whhT"],
        }
        m.update(common)
        in_maps.append(m)
    return in_maps


def _run(nc, in_maps, trace=False, trace_kwargs=None):
    from concourse.bass_utils import run_bass_kernel_spmd

    last_err = None
    for _ in range(3):
        try:
            return run_bass_kernel_spmd(nc, in_maps,
                                        core_ids=list(range(NCORES)),
                                        trace=trace,
                                        **(trace_kwargs or {}))
        except Exception as e:  # transient NRT device errors
            last_err = e
            import time
            time.sleep(5)
    raise last_err


def kernel(**inputs):
    T = np.asarray(inputs["x"]).shape[1]
    if T not in _PROGRAM_CACHE:
        _PROGRAM_CACHE[T] = _build(T)
    nc = _PROGRAM_CACHE[T]
    in_maps = _make_in_maps(inputs, T)
    res = _run(nc, in_maps)
    out = np.zeros((B, C), dtype=np.float32)
    for p in range(4):
        out[8 * p:8 * p + 8, :] = res.results[p]["outT"].T
    return out


# revision 10
# speedup vs baseline: 2.6101x; 2.6101x over previous
"""Bidirectional 2-layer GRU + FC kernel for Trainium2 (8 NeuronCores).

Strategy:
  - Cores 0-3 run layer-0 FORWARD for batch slice [8p, 8p+8); cores 4-7 run
    layer-0 BACKWARD for the same slices (fed time-reversed x + backward
    weights via per-core in_maps; the device program is SPMD-uniform).
  - gx (input projections + biases) are precomputed with big fp32r matmuls.
  - The recurrence runs in transposed layout: gate rows on partitions,
    batch on the free dim.  Per step: 48 fp32r matmuls (w_hh.T stationary
    chunks) + K=1 bias matmuls, then sigmoid/tanh on ScalarE and
    elementwise on VectorE.
  - A pairwise AllGather {i, i+4} shares the layer-0 trajectories; both pair
    members then redundantly compute layer-1 forward for their 8 batches
    (recurrence wall-time is batch-independent, so redundancy is free).
  - Layer-1 backward contributes only its t=T-1 state to the output, which
    takes a single step from h0=0.  FC bias is fused into an ACT Identity.
"""

import contextlib

import numpy as np

B, T_FULL, I_IN, H, C = 32, 512, 256, 512, 10
NCORES = 8
BA = 8          # batch per core
MCH = 12        # 3H / 128 gate-row chunks
KH = 4          # H / 128 contraction chunks

_PROGRAM_CACHE = {}


def _build(T):
    import concourse.bacc as bacc
    import concourse.mybir as mybir
    import concourse.tile as tile

    f32 = mybir.dt.float32
    f32r = mybir.dt.float32r
    bf16 = mybir.dt.bfloat16
    SIG = mybir.ActivationFunctionType.Sigmoid
    TANH = mybir.ActivationFunctionType.Tanh
    IDENT = mybir.ActivationFunctionType.Identity
    ALU = mybir.AluOpType

    TB = min(64, T)          # timestep block for the big matmul phases
    NTB = T // TB

    nc = bacc.Bacc("TRN2", target_bir_lowering=False, debug=False,
                   num_devices=NCORES)

    def inp(name, shape, dt=f32r):
        return nc.dram_tensor(name, shape, dt, kind="ExternalInput").ap()

    xT = inp("xT", [I_IN, T, BA])               # own batch slice, own time order
    wihT0 = inp("wihT0", [I_IN, 3 * H])         # own direction w_ih.T
    bias0 = inp("bias0", [1, 3 * H])            # b_ih + b_hh (rz); n part = b_ih_n
    bhn0 = inp("bhn0", [1, H], bf16)                  # b_hh n part
    whhT0 = inp("whhT0", [H, 3 * H], bf16)
    wih1T_f = inp("wih1T_f", [H, 3 * H], bf16)        # w_ih_l1f.T rows 0:H   (f0 input)
    wih1T_b = inp("wih1T_b", [H, 3 * H], bf16)        # w_ih_l1f.T rows H:2H  (b0 input)
    bias1 = inp("bias1", [1, 3 * H])
    bhn1 = inp("bhn1", [1, H], bf16)
    whh1T = inp("whh1T", [H, 3 * H], bf16)
    wih1bT = inp("wih1bT", [2 * H, 3 * H], bf16)      # w_ih_l1b.T
    bias1b_sc = inp("bias1b_sc", [128, MCH], f32)   # per m-chunk column
    bhn1b_sc = inp("bhn1b_sc", [128, KH], f32)
    fcwT = inp("fcwT", [2 * H, C])
    fcb = inp("fcb", [C, 1], f32)

    outT = nc.dram_tensor("outT", [C, BA], f32, kind="ExternalOutput").ap()

    with tile.TileContext(nc) as tc, contextlib.ExitStack() as ctx:
        # ---------------- DRAM scratch (Tile-tracked) ----------------
        dramp = ctx.enter_context(tc.tile_pool(name="dramp", bufs=1, space="DRAM"))
        gx0 = dramp.tile([T, 128, MCH * BA], f32, tag="gx0")
        hbuf = dramp.tile([T, 128, KH * BA], bf16, tag="hbuf")
        agbuf = dramp.tile([2, T, 128, KH * BA], bf16, tag="agbuf")
        gx1f = dramp.tile([T, 128, MCH * BA], f32, tag="gx1f")
        gx1b = dramp.tile([T, 128, MCH * BA], f32, tag="gx1b")

        # ---------------- persistent SBUF (one pool, distinct tags) --------
        constp = ctx.enter_context(tc.tile_pool(name="constp", bufs=1))

        def const_tile(shape, dt, tag):
            return constp.tile(shape, dt, tag=tag, name=tag)

        whhT0_sb = const_tile([128, KH, 3 * H], bf16, "whhT0_sb")
        nc.sync.dma_start(whhT0_sb[:], whhT0.rearrange("(k p) m -> p k m", p=128))
        bias0_sb = const_tile([128, 3 * H], f32r, "bias0_sb")[0:1, :]
        nc.sync.dma_start(bias0_sb, bias0[:])
        bhn0_sb = const_tile([128, H], bf16, "bhn0_sb")[0:1, :]
        nc.sync.dma_start(bhn0_sb, bhn0[:])
        ones_big = const_tile([128, TB * BA], f32, "ones_big")[0:1, :]
        nc.vector.memset(ones_big, 1.0)
        ones_bf = const_tile([128, BA], bf16, "ones_bf")[0:1, :]
        nc.vector.memset(ones_bf, 1.0)

        whh1_sb = const_tile([128, KH, 3 * H], bf16, "whh1_sb")
        nc.sync.dma_start(whh1_sb[:], whh1T.rearrange("(k p) m -> p k m", p=128))
        bhn1_sb = const_tile([128, H], bf16, "bhn1_sb")[0:1, :]
        nc.sync.dma_start(bhn1_sb, bhn1[:])
        bias1_sb = const_tile([128, 3 * H], f32r, "bias1_sb")[0:1, :]
        nc.sync.dma_start(bias1_sb, bias1[:])

        b1b_sb = const_tile([128, MCH], f32, "b1b_sb")
        nc.sync.dma_start(b1b_sb[:], bias1b_sc[:])
        bhn1b_sb = const_tile([128, KH], f32, "bhn1b_sb")
        nc.sync.dma_start(bhn1b_sb[:], bhn1b_sc[:])
        fcw_sb = const_tile([128, 2 * KH, C], f32r, "fcw_sb")
        nc.sync.dma_start(fcw_sb[:], fcwT.rearrange("(k p) c -> p k c", p=128))
        fcb_sb = const_tile([128, 1], f32, "fcb_sb")[0:C, :]
        nc.sync.dma_start(fcb_sb, fcb[:])

        # h-state tiles for both scans + l1b results (long-lived)
        h0a = const_tile([128, KH * BA], bf16, "h0a")
        h0b = const_tile([128, KH * BA], bf16, "h0b")
        h1a = const_tile([128, KH * BA], bf16, "h1a")
        h1b_ = const_tile([128, KH * BA], bf16, "h1b_")
        gxl = const_tile([128, MCH * BA], f32, "gxl")
        rl = const_tile([128, 4 * BA], f32, "rl")
        zpl = const_tile([128, 4 * BA], f32, "zpl")
        n1l = const_tile([128, 4 * BA], f32, "n1l")
        ntl = const_tile([128, 4 * BA], f32, "ntl")
        h1bk = const_tile([128, 4 * BA], f32r, "h1bk")
        x1last = const_tile([128, 2 * KH, BA], bf16, "x1last")
        out_sb = const_tile([128, BA], f32, "out_sb")[0:C, :]

        # ================ Phase 0: gx0 = w_ih0 @ x.T + bias0 ================
        with contextlib.ExitStack() as p0s:
            wp0 = p0s.enter_context(tc.tile_pool(name="wp0", bufs=1))
            wihT0_sb = wp0.tile([128, 2, 3 * H], f32r, tag="wihT0_sb")
            nc.sync.dma_start(wihT0_sb[:],
                              wihT0.rearrange("(k p) m -> p k m", p=128))
            p0_mv = p0s.enter_context(tc.tile_pool(name="p0_mv", bufs=3))
            p0_ps = p0s.enter_context(tc.tile_pool(name="p0_ps", bufs=4,
                                                   space="PSUM"))
            p0_out = p0s.enter_context(tc.tile_pool(name="p0_out", bufs=4))

            xT_r = xT.rearrange("(k p) t b -> p k t b", p=128)
            for c in range(NTB):
                mv = p0_mv.tile([128, 2, TB, BA], f32r, tag="mv")
                nc.sync.dma_start(mv[:], xT_r[:, :, c * TB:(c + 1) * TB, :])
                for m in range(MCH):
                    ps = p0_ps.tile([128, TB * BA], f32, tag="ps")
                    for k in range(2):
                        nc.tensor.matmul(ps[:],
                                         wihT0_sb[:, k, 128 * m:128 * (m + 1)],
                                         mv[:, k, :, :],
                                         start=(k == 0), stop=False)
                    nc.tensor.matmul(ps[:], bias0_sb[:, 128 * m:128 * (m + 1)],
                                     ones_big.bitcast(f32r),
                                     start=False, stop=True)
                    ot = p0_out.tile([128, TB, BA], f32, tag="ot")
                    flat = ot[:].rearrange("p t b -> p (t b)")
                    if m % 2 == 0:
                        nc.vector.tensor_copy(flat, ps[:])
                    else:
                        nc.scalar.copy(flat, ps[:])
                    nc.scalar.dma_start(
                        gx0[c * TB:(c + 1) * TB, :, BA * m:BA * (m + 1)]
                        .transpose([1, 0, 2]), ot[:])

        # ================ generic GRU scan ================
        def scan(h_even, h_odd, gx_load, whh_sb, bhn_sb, store_h, name):
            h_t = [h_even, h_odd]
            nc.vector.memset(h_t[0][:], 0.0)
            with contextlib.ExitStack() as ss:
                ps_rz_p = ss.enter_context(
                    tc.tile_pool(name=f"{name}_psrz", bufs=2, space="PSUM"))
                ps_n_p = ss.enter_context(
                    tc.tile_pool(name=f"{name}_psn", bufs=2, space="PSUM"))
                gxp = ss.enter_context(tc.tile_pool(name=f"{name}_gx", bufs=4))
                gp = ss.enter_context(tc.tile_pool(name=f"{name}_g", bufs=3))

                for t in range(T):
                    h_cur, h_nxt = h_t[t % 2], h_t[(t + 1) % 2]
                    gx = gx_load(t, gxp)
                    ps_rz = ps_rz_p.tile([128, 8 * BA], f32, tag="psrz")
                    ps_n = ps_n_p.tile([128, 4 * BA], f32, tag="psn")
                    hr = h_cur[:]
                    for m in range(8):
                        for k in range(KH):
                            nc.tensor.matmul(ps_rz[:, BA * m:BA * (m + 1)],
                                             whh_sb[:, k, 128 * m:128 * (m + 1)],
                                             hr[:, BA * k:BA * (k + 1)],
                                             start=(k == 0), stop=(k == KH - 1))
                    for m in range(8, MCH):
                        mm = m - 8
                        for k in range(KH):
                            nc.tensor.matmul(ps_n[:, BA * mm:BA * (mm + 1)],
                                             whh_sb[:, k, 128 * m:128 * (m + 1)],
                                             hr[:, BA * k:BA * (k + 1)],
                                             start=(k == 0), stop=False)
                        nc.tensor.matmul(ps_n[:, BA * mm:BA * (mm + 1)],
                                         bhn_sb[:, 128 * mm:128 * (mm + 1)],
                                         ones_bf,
                                         start=False, stop=True)
                    rz_pre = gp.tile([128, 8 * BA], f32, tag="rz_pre")
                    nc.vector.tensor_add(rz_pre[:], ps_rz[:], gx[:, 0:8 * BA])
                    rz = gp.tile([128, 8 * BA], f32, tag="rz")
                    nc.scalar.activation(rz[:], rz_pre[:], SIG)
                    n1 = gp.tile([128, 4 * BA], f32, tag="n1")
                    nc.vector.tensor_mul(n1[:], ps_n[:], rz[:, 0:4 * BA])
                    n2 = gp.tile([128, 4 * BA], f32, tag="n2")
                    nc.vector.tensor_add(n2[:], n1[:], gx[:, 8 * BA:MCH * BA])
                    nt = gp.tile([128, 4 * BA], f32, tag="nt")
                    nc.scalar.activation(nt[:], n2[:], TANH)
                    d = gp.tile([128, 4 * BA], f32, tag="d")
                    nc.vector.tensor_sub(d[:], h_cur[:], nt[:])
                    e = gp.tile([128, 4 * BA], f32, tag="e")
                    nc.vector.tensor_mul(e[:], d[:], rz[:, 4 * BA:8 * BA])
                    nc.vector.tensor_add(h_nxt[:], e[:], nt[:])
                    if store_h is not None:
                        nc.scalar.dma_start(store_h[t], h_nxt[:])
            return h_t[T % 2]

        # ================ Phase A: layer-0 scan (own direction) ============
        def gx0_load(t, pool):
            g = pool.tile([128, MCH * BA], f32, tag="gx")
            nc.sync.dma_start(g[:], gx0[t])
            return g

        scan(h0a, h0b, gx0_load, whhT0_sb, bhn0_sb, hbuf, "s0")

        # ================ AllGather pair {i, i+4} ================
        nc.gpsimd.collective_compute(
            "AllGather", ALU.bypass,
            replica_groups=[[0, 4], [1, 5], [2, 6], [3, 7]],
            ins=[hbuf[:]], outs=[agbuf[:]])

        # ================ Phase gx1: two passes over agbuf ================
        with contextlib.ExitStack() as g1s:
            w1p = g1s.enter_context(tc.tile_pool(name="w1p", bufs=2))
            g1_mv = g1s.enter_context(tc.tile_pool(name="g1_mv", bufs=3))
            g1_ps = g1s.enter_context(tc.tile_pool(name="g1_ps", bufs=4,
                                                   space="PSUM"))
            g1_out = g1s.enter_context(tc.tile_pool(name="g1_out", bufs=4))
            for pi, (wsrc, dst, with_bias) in enumerate(
                    [(wih1T_f, gx1f, True), (wih1T_b, gx1b, False)]):
                w_sb = w1p.tile([128, KH, 3 * H], bf16, tag="w1")
                nc.sync.dma_start(w_sb[:],
                                  wsrc.rearrange("(k p) m -> p k m", p=128))
                for c in range(NTB):
                    mv = g1_mv.tile([128, KH, TB, BA], bf16, tag="g1mv")
                    for j in range(KH):
                        nc.sync.dma_start(
                            mv[:, j, :, :],
                            agbuf[pi, c * TB:(c + 1) * TB, :,
                                  BA * j:BA * (j + 1)].transpose([1, 0, 2]))
                    for m in range(MCH):
                        ps = g1_ps.tile([128, TB * BA], f32, tag="ps")
                        for k in range(KH):
                            nc.tensor.matmul(
                                ps[:], w_sb[:, k, 128 * m:128 * (m + 1)],
                                mv[:, k, :, :],
                                start=(k == 0),
                                stop=(not with_bias and k == KH - 1))
                        if with_bias:
                            nc.tensor.matmul(
                                ps[:], bias1_sb[:, 128 * m:128 * (m + 1)],
                                ones_big.bitcast(f32r), start=False, stop=True)
                        ot = g1_out.tile([128, TB, BA], f32, tag="ot")
                        flat = ot[:].rearrange("p t b -> p (t b)")
                        if m % 2 == 0:
                            nc.vector.tensor_copy(flat, ps[:])
                        else:
                            nc.scalar.copy(flat, ps[:])
                        nc.scalar.dma_start(
                            dst[c * TB:(c + 1) * TB, :, BA * m:BA * (m + 1)]
                            .transpose([1, 0, 2]), ot[:])

        # ================ layer-1 backward: single step at t = T-1 =========
        with contextlib.ExitStack() as lbs:
            lb_wp = lbs.enter_context(tc.tile_pool(name="lb_wp", bufs=1))
            l1b_w = lb_wp.tile([128, 2 * KH, 3 * H], bf16, tag="l1b_w")
            nc.sync.dma_start(l1b_w[:],
                              wih1bT.rearrange("(k p) m -> p k m", p=128))
            for j in range(KH):
                nc.sync.dma_start(x1last[:, j, :],
                                  agbuf[0, T - 1, :, BA * j:BA * (j + 1)])
                nc.sync.dma_start(x1last[:, KH + j, :],
                                  agbuf[1, 0, :, BA * j:BA * (j + 1)])
            l1b_ps_p = lbs.enter_context(
                tc.tile_pool(name="l1b_ps", bufs=1, space="PSUM"))
            l1b_ps = l1b_ps_p.tile([128, MCH * BA], f32, tag="l1b_ps")
            for m in range(MCH):
                for k in range(2 * KH):
                    nc.tensor.matmul(l1b_ps[:, BA * m:BA * (m + 1)],
                                     l1b_w[:, k, 128 * m:128 * (m + 1)],
                                     x1last[:, k, :],
                                     start=(k == 0), stop=(k == 2 * KH - 1))
            for m in range(MCH):
                nc.vector.tensor_scalar_add(gxl[:, BA * m:BA * (m + 1)],
                                            l1b_ps[:, BA * m:BA * (m + 1)],
                                            b1b_sb[:, m:m + 1])
            nc.scalar.activation(rl[:], gxl[:, 0:4 * BA], SIG)
            nc.scalar.activation(zpl[:], gxl[:, 4 * BA:8 * BA], SIG, scale=-1.0)
            for jj in range(KH):
                nc.vector.scalar_tensor_tensor(
                    n1l[:, BA * jj:BA * (jj + 1)], rl[:, BA * jj:BA * (jj + 1)],
                    bhn1b_sb[:, jj:jj + 1],
                    gxl[:, 8 * BA + BA * jj:8 * BA + BA * (jj + 1)],
                    ALU.mult, ALU.add)
            nc.scalar.activation(ntl[:], n1l[:], TANH)
            nc.vector.tensor_mul(h1bk[:], zpl[:], ntl[:])

        # ================ Phase B: layer-1 forward scan ================
        def gx1_load(t, pool):
            ga = pool.tile([128, MCH * BA], f32, tag="gxa")
            nc.sync.dma_start(ga[:], gx1f[t])
            gb = pool.tile([128, MCH * BA], f32, tag="gxb")
            nc.sync.dma_start(gb[:], gx1b[T - 1 - t])
            gs = pool.tile([128, MCH * BA], f32, tag="gxs")
            nc.vector.tensor_add(gs[:], ga[:], gb[:])
            return gs

        h1f_bf = scan(h1a, h1b_, gx1_load, whh1_sb, bhn1_sb, None, "s1")
        h1f = const_tile([128, KH * BA], f32r, "h1f_r")
        nc.vector.tensor_copy(h1f[:], h1f_bf[:])

        # ================ FC ================
        with contextlib.ExitStack() as fcs:
            fc_ps_p = fcs.enter_context(
                tc.tile_pool(name="fc_ps", bufs=1, space="PSUM"))
            fc_ps_t = fc_ps_p.tile([128, BA], f32, tag="fc_ps", name="fc_ps")
            fc_ps = fc_ps_t[0:C, :]
            for k in range(KH):
                nc.tensor.matmul(fc_ps, fcw_sb[:, k, :],
                                 h1f[:, BA * k:BA * (k + 1)],
                                 start=(k == 0), stop=False)
            for k in range(KH):
                nc.tensor.matmul(fc_ps, fcw_sb[:, KH + k, :],
                                 h1bk[:, BA * k:BA * (k + 1)],
                                 start=False, stop=(k == KH - 1))
            nc.scalar.activation(out_sb, fc_ps, IDENT, bias=fcb_sb)
            nc.sync.dma_start(outT[:], out_sb)

    nc.compile()
    return nc


def _make_in_maps(inputs, T):
    x = np.asarray(inputs["x"], dtype=np.float32)

    import ml_dtypes
    bf = ml_dtypes.bfloat16

    def layer_params(wih, whh, bih, bhh):
        wih, whh = np.asarray(wih), np.asarray(whh)
        bih, bhh = np.asarray(bih), np.asarray(bhh)
        bias = (bih + bhh).astype(np.float32).copy()
        bias[2 * H:] = bih[2 * H:]
        return {
            "wihT": np.ascontiguousarray(wih.T, dtype=np.float32),
            "whhT": np.ascontiguousarray(whh.T).astype(bf),
            "bias": bias.reshape(1, 3 * H),
            "bhn": bhh[2 * H:].reshape(1, H).astype(bf),
        }

    l0f = layer_params(inputs["w_ih_l0f"], inputs["w_hh_l0f"],
                       inputs["b_ih_l0f"], inputs["b_hh_l0f"])
    l0b = layer_params(inputs["w_ih_l0b"], inputs["w_hh_l0b"],
                       inputs["b_ih_l0b"], inputs["b_hh_l0b"])
    l1f = layer_params(inputs["w_ih_l1f"], inputs["w_hh_l1f"],
                       inputs["b_ih_l1f"], inputs["b_hh_l1f"])

    wih1fT = np.ascontiguousarray(np.asarray(inputs["w_ih_l1f"]).T
                                  ).astype(bf)  # [2H, 3H]
    wih1bT = np.ascontiguousarray(np.asarray(inputs["w_ih_l1b"]).T).astype(bf)

    b1b = (np.asarray(inputs["b_ih_l1b"]) + np.asarray(inputs["b_hh_l1b"])
           ).astype(np.float32).copy()
    b1b[2 * H:] = np.asarray(inputs["b_ih_l1b"])[2 * H:]
    bias1b_sc = np.ascontiguousarray(b1b.reshape(MCH, 128).T)
    bhn1b_sc = np.ascontiguousarray(
        np.asarray(inputs["b_hh_l1b"])[2 * H:].reshape(KH, 128).T
        .astype(np.float32))

    fcwT = np.ascontiguousarray(np.asarray(inputs["fc_w"]).T, dtype=np.float32)
    fcb = np.asarray(inputs["fc_b"]).reshape(C, 1).astype(np.float32)

    common = {
        "wih1T_f": np.ascontiguousarray(wih1fT[:H]),
        "wih1T_b": np.ascontiguousarray(wih1fT[H:]),
        "bias1": l1f["bias"],
        "bhn1": l1f["bhn"],
        "whh1T": l1f["whhT"],
        "wih1bT": wih1bT,
        "bias1b_sc": bias1b_sc,
        "bhn1b_sc": bhn1b_sc,
        "fcwT": fcwT,
        "fcb": fcb,
    }

    in_maps = []
    for i in range(NCORES):
        p = i % 4
        back = i >= 4
        xs = x[8 * p:8 * p + 8, :T, :]
        if back:
            xs = xs[:, ::-1, :]
        xTl = np.ascontiguousarray(xs.transpose(2, 1, 0))  # [I, T, BA]
        lp = l0b if back else l0f
        m = {
            "xT": xTl,
            "wihT0": lp["wihT"],
            "bias0": lp["bias"],
            "bhn0": lp["bhn"],
            "whhT0": lp["whhT"],
        }
        m.update(common)
        in_maps.append(m)
    return in_maps


def _run(nc, in_maps, trace=False, trace_kwargs=None):
    from concourse.bass_utils import run_bass_kernel_spmd

    last_err = None
    for _ in range(3):
        try:
            return run_bass_kernel_spmd(nc, in_maps,
                                        core_ids=list(range(NCORES)),
                                        trace=trace,
                                        **(trace_kwargs or {}))
        except Exception as e:  # transient NRT device errors
            last_err = e
            import time
            time.sleep(5)
    raise last_err


def kernel(**inputs):
    T = np.asarray(inputs["x"]).shape[1]
    if T not in _PROGRAM_CACHE:
        _PROGRAM_CACHE[T] = _build(T)
    nc = _PROGRAM_CACHE[T]
    in_maps = _make_in_maps(inputs, T)
    res = _run(nc, in_maps)
    out = np.zeros((B, C), dtype=np.float32)
    for p in range(4):
        out[8 * p:8 * p + 8, :] = res.results[p]["outT"].T
    return out


# revision 12
# speedup vs baseline: 3.2848x; 1.2585x over previous
"""Bidirectional 2-layer GRU + FC kernel for Trainium2 (8 NeuronCores).

Strategy:
  - Cores 0-3 run layer-0 FORWARD for batch slice [8p, 8p+8); cores 4-7 run
    layer-0 BACKWARD for the same slices (fed time-reversed x + backward
    weights via per-core in_maps; the device program is SPMD-uniform).
  - gx (input projections + biases) are precomputed with big fp32r matmuls.
  - The recurrence runs in transposed layout: gate rows on partitions,
    batch on the free dim.  Per step: 48 fp32r matmuls (w_hh.T stationary
    chunks) + K=1 bias matmuls, then sigmoid/tanh on ScalarE and
    elementwise on VectorE.
  - A pairwise AllGather {i, i+4} shares the layer-0 trajectories; both pair
    members then redundantly compute layer-1 forward for their 8 batches
    (recurrence wall-time is batch-independent, so redundancy is free).
  - Layer-1 backward contributes only its t=T-1 state to the output, which
    takes a single step from h0=0.  FC bias is fused into an ACT Identity.
"""

import contextlib

import numpy as np

B, T_FULL, I_IN, H, C = 32, 512, 256, 512, 10
NCORES = 8
BA = 8          # batch per core
MCH = 12        # 3H / 128 gate-row chunks
KH = 4          # H / 128 contraction chunks

_PROGRAM_CACHE = {}


def _build(T):
    import concourse.bacc as bacc
    import concourse.mybir as mybir
    import concourse.tile as tile

    f32 = mybir.dt.float32
    f32r = mybir.dt.float32r
    bf16 = mybir.dt.bfloat16
    SIG = mybir.ActivationFunctionType.Sigmoid
    TANH = mybir.ActivationFunctionType.Tanh
    IDENT = mybir.ActivationFunctionType.Identity
    ALU = mybir.AluOpType

    TB = min(64, T)          # timestep block for the big matmul phases
    NTB = T // TB

    nc = bacc.Bacc("TRN2", target_bir_lowering=False, debug=False,
                   num_devices=NCORES)

    def inp(name, shape, dt=f32r):
        return nc.dram_tensor(name, shape, dt, kind="ExternalInput").ap()

    xT = inp("xT", [I_IN, T, BA])               # own batch slice, own time order
    wihT0 = inp("wihT0", [I_IN, 3 * H])         # own direction w_ih.T
    bias0 = inp("bias0", [1, 3 * H])            # b_ih + b_hh (rz); n part = b_ih_n
    bhn0 = inp("bhn0", [1, H], bf16)                  # b_hh n part
    whhT0 = inp("whhT0", [H, 3 * H], bf16)
    wih1T_f = inp("wih1T_f", [H, 3 * H], bf16)        # w_ih_l1f.T rows 0:H   (f0 input)
    wih1T_b = inp("wih1T_b", [H, 3 * H], bf16)        # w_ih_l1f.T rows H:2H  (b0 input)
    bias1 = inp("bias1", [1, 3 * H])
    bhn1 = inp("bhn1", [1, H], bf16)
    whh1T = inp("whh1T", [H, 3 * H], bf16)
    wih1bT = inp("wih1bT", [2 * H, 3 * H], bf16)      # w_ih_l1b.T
    bias1b_sc = inp("bias1b_sc", [128, MCH], f32)   # per m-chunk column
    bhn1b_sc = inp("bhn1b_sc", [128, KH], f32)
    fcwT = inp("fcwT", [2 * H, C])
    fcb = inp("fcb", [C, 1], f32)

    outT = nc.dram_tensor("outT", [C, BA], f32, kind="ExternalOutput").ap()

    with tile.TileContext(nc) as tc, contextlib.ExitStack() as ctx:
        # ---------------- DRAM scratch (Tile-tracked) ----------------
        dramp = ctx.enter_context(tc.tile_pool(name="dramp", bufs=1, space="DRAM"))
        gx0 = dramp.tile([128, MCH, T, BA], f32, tag="gx0")
        hbuf = dramp.tile([128, T, KH * BA], bf16, tag="hbuf")
        agbuf = dramp.tile([2, 128, T, KH * BA], bf16, tag="agbuf")
        gx1f = dramp.tile([128, MCH, T, BA], f32, tag="gx1f")
        gx1b = dramp.tile([128, MCH, T, BA], f32, tag="gx1b")

        # ---------------- persistent SBUF (one pool, distinct tags) --------
        constp = ctx.enter_context(tc.tile_pool(name="constp", bufs=1))

        def const_tile(shape, dt, tag):
            return constp.tile(shape, dt, tag=tag, name=tag)

        whhT0_sb = const_tile([128, KH, 3 * H], bf16, "whhT0_sb")
        nc.sync.dma_start(whhT0_sb[:], whhT0.rearrange("(k p) m -> p k m", p=128))
        bias0_sb = const_tile([128, 3 * H], f32r, "bias0_sb")[0:1, :]
        nc.sync.dma_start(bias0_sb, bias0[:])
        bhn0_sb = const_tile([128, H], bf16, "bhn0_sb")[0:1, :]
        nc.sync.dma_start(bhn0_sb, bhn0[:])
        ones_big = const_tile([128, TB * BA], f32, "ones_big")[0:1, :]
        nc.vector.memset(ones_big, 1.0)
        ones_bf = const_tile([128, BA], bf16, "ones_bf")[0:1, :]
        nc.vector.memset(ones_bf, 1.0)

        whh1_sb = const_tile([128, KH, 3 * H], bf16, "whh1_sb")
        nc.sync.dma_start(whh1_sb[:], whh1T.rearrange("(k p) m -> p k m", p=128))
        bhn1_sb = const_tile([128, H], bf16, "bhn1_sb")[0:1, :]
        nc.sync.dma_start(bhn1_sb, bhn1[:])
        bias1_sb = const_tile([128, 3 * H], f32r, "bias1_sb")[0:1, :]
        nc.sync.dma_start(bias1_sb, bias1[:])

        b1b_sb = const_tile([128, MCH], f32, "b1b_sb")
        nc.sync.dma_start(b1b_sb[:], bias1b_sc[:])
        bhn1b_sb = const_tile([128, KH], f32, "bhn1b_sb")
        nc.sync.dma_start(bhn1b_sb[:], bhn1b_sc[:])
        fcw_sb = const_tile([128, 2 * KH, C], f32r, "fcw_sb")
        nc.sync.dma_start(fcw_sb[:], fcwT.rearrange("(k p) c -> p k c", p=128))
        fcb_sb = const_tile([128, 1], f32, "fcb_sb")[0:C, :]
        nc.sync.dma_start(fcb_sb, fcb[:])

        # h-state tiles for both scans + l1b results (long-lived)
        h0a = const_tile([128, KH * BA], bf16, "h0a")
        h0b = const_tile([128, KH * BA], bf16, "h0b")
        h1a = const_tile([128, KH * BA], bf16, "h1a")
        h1b_ = const_tile([128, KH * BA], bf16, "h1b_")
        gxl = const_tile([128, MCH * BA], f32, "gxl")
        rl = const_tile([128, 4 * BA], f32, "rl")
        zpl = const_tile([128, 4 * BA], f32, "zpl")
        n1l = const_tile([128, 4 * BA], f32, "n1l")
        ntl = const_tile([128, 4 * BA], f32, "ntl")
        h1bk = const_tile([128, 4 * BA], f32r, "h1bk")
        x1last = const_tile([128, 2 * KH, BA], bf16, "x1last")
        out_sb = const_tile([128, BA], f32, "out_sb")[0:C, :]

        # ================ Phase 0: gx0 = w_ih0 @ x.T + bias0 ================
        with contextlib.ExitStack() as p0s:
            wp0 = p0s.enter_context(tc.tile_pool(name="wp0", bufs=1))
            wihT0_sb = wp0.tile([128, 2, 3 * H], f32r, tag="wihT0_sb")
            nc.sync.dma_start(wihT0_sb[:],
                              wihT0.rearrange("(k p) m -> p k m", p=128))
            p0_mv = p0s.enter_context(tc.tile_pool(name="p0_mv", bufs=3))
            p0_ps = p0s.enter_context(tc.tile_pool(name="p0_ps", bufs=4,
                                                   space="PSUM"))
            p0_out = p0s.enter_context(tc.tile_pool(name="p0_out", bufs=4))

            xT_r = xT.rearrange("(k p) t b -> p k t b", p=128)
            for c in range(NTB):
                mv = p0_mv.tile([128, 2, TB, BA], f32r, tag="mv")
                nc.sync.dma_start(mv[:], xT_r[:, :, c * TB:(c + 1) * TB, :])
                for m in range(MCH):
                    ps = p0_ps.tile([128, TB * BA], f32, tag="ps")
                    for k in range(2):
                        nc.tensor.matmul(ps[:],
                                         wihT0_sb[:, k, 128 * m:128 * (m + 1)],
                                         mv[:, k, :, :],
                                         start=(k == 0), stop=False)
                    nc.tensor.matmul(ps[:], bias0_sb[:, 128 * m:128 * (m + 1)],
                                     ones_big.bitcast(f32r),
                                     start=False, stop=True)
                    ot = p0_out.tile([128, TB, BA], f32, tag="ot")
                    flat = ot[:].rearrange("p t b -> p (t b)")
                    if m % 2 == 0:
                        nc.vector.tensor_copy(flat, ps[:])
                    else:
                        nc.scalar.copy(flat, ps[:])
                    nc.scalar.dma_start(
                        gx0[:, m, c * TB:(c + 1) * TB, :], ot[:])

        # ================ generic GRU scan (half-split pipeline) ============
        # Halves over H-chunks: X=A -> chunks {0,1} (m in 0,1/4,5/8,9),
        # X=B -> chunks {2,3}.  Gate math for half A overlaps PE work for
        # half B and the next step's K-chunks 0,1.
        def scan(h_even, h_odd, gx_load, whh_sb, bhn_sb, store_h, name):
            h_t = [h_even, h_odd]
            nc.vector.memset(h_t[0][:], 0.0)
            with contextlib.ExitStack() as ss:
                ps_p = [ss.enter_context(tc.tile_pool(
                            name=f"{name}_ps{x}", bufs=2, space="PSUM"))
                        for x in range(2)]
                gxp = ss.enter_context(tc.tile_pool(name=f"{name}_gx", bufs=6))
                gp = ss.enter_context(tc.tile_pool(name=f"{name}_g", bufs=3))
                HB = 2 * BA  # half width in cols (2 H-chunks x BA)

                for t in range(T):
                    h_cur, h_nxt = h_t[t % 2], h_t[(t + 1) % 2]
                    gx = gx_load(t, gxp)
                    # gx grouped [128, 3 gates, 4 chunks * BA]
                    gxg = gx[:].rearrange("p (g x) -> p g x", g=3)
                    ps_h = []
                    for half in range(2):
                        ps = ps_p[half].tile([128, 6 * BA], f32, tag="ps",
                                             name=f"{name}_ps_t")
                        ps_h.append(ps)
                        c0 = 2 * half  # first H-chunk of this half
                        for gi in range(3):          # r, z, n gate groups
                            for mm in (0, 1):        # chunk within half
                                m = 4 * gi + c0 + mm
                                dst = ps[:, BA * (2 * gi + mm):
                                         BA * (2 * gi + mm + 1)]
                                for k in range(KH):
                                    nc.tensor.matmul(
                                        dst, whh_sb[:, k, 128 * m:128 * (m + 1)],
                                        h_cur[:, BA * k:BA * (k + 1)],
                                        start=(k == 0),
                                        stop=(gi < 2 and k == KH - 1))
                                if gi == 2:
                                    nc.tensor.matmul(
                                        dst, bhn_sb[:, 128 * (c0 + mm):
                                                    128 * (c0 + mm + 1)],
                                        ones_bf, start=False, stop=True)
                    for half in range(2):
                        ps = ps_h[half]
                        c0 = 2 * half
                        gx_rz = gxg[:, 0:2, HB * half:HB * (half + 1)]
                        gx_n = gxg[:, 2, HB * half:HB * (half + 1)]
                        rz_pre = gp.tile([128, 2, HB], f32, tag="rz_pre",
                                         name=f"{name}_rzp")
                        nc.vector.tensor_add(
                            rz_pre[:], ps[:, 0:2 * HB]
                            .rearrange("p (g x) -> p g x", g=2), gx_rz)
                        rz = gp.tile([128, 2 * HB], f32, tag="rz",
                                     name=f"{name}_rz")
                        nc.scalar.activation(rz[:], rz_pre[:]
                                             .rearrange("p g x -> p (g x)"), SIG)
                        n1 = gp.tile([128, HB], f32, tag="n1", name=f"{name}_n1")
                        nc.vector.tensor_mul(n1[:], ps[:, 2 * HB:3 * HB],
                                             rz[:, 0:HB])
                        n2 = gp.tile([128, HB], f32, tag="n2", name=f"{name}_n2")
                        nc.vector.tensor_add(n2[:], n1[:], gx_n)
                        nt = gp.tile([128, HB], f32, tag="nt", name=f"{name}_nt")
                        nc.scalar.activation(nt[:], n2[:], TANH)
                        d = gp.tile([128, HB], f32, tag="d", name=f"{name}_d")
                        nc.vector.tensor_sub(d[:], h_cur[:, HB * half:
                                                         HB * (half + 1)], nt[:])
                        e = gp.tile([128, HB], f32, tag="e", name=f"{name}_e")
                        nc.vector.tensor_mul(e[:], d[:], rz[:, HB:2 * HB])
                        nc.vector.tensor_add(h_nxt[:, HB * half:HB * (half + 1)],
                                             e[:], nt[:])
                    if store_h is not None:
                        nc.scalar.dma_start(store_h[:, t, :], h_nxt[:])
            return h_t[T % 2]

        # ================ Phase A: layer-0 scan (own direction) ============
        def gx0_load(t, pool):
            g = pool.tile([128, MCH * BA], f32, tag="gx")
            nc.sync.dma_start(g[:].rearrange("p (m b) -> p m b", m=MCH),
                              gx0[:, :, t, :])
            return g

        scan(h0a, h0b, gx0_load, whhT0_sb, bhn0_sb, hbuf, "s0")

        # ================ AllGather pair {i, i+4} ================
        nc.gpsimd.collective_compute(
            "AllGather", ALU.bypass,
            replica_groups=[[0, 4], [1, 5], [2, 6], [3, 7]],
            ins=[hbuf[:]], outs=[agbuf[:]])

        # ================ Phase gx1: two passes over agbuf ================
        with contextlib.ExitStack() as g1s:
            w1p = g1s.enter_context(tc.tile_pool(name="w1p", bufs=2))
            g1_mv = g1s.enter_context(tc.tile_pool(name="g1_mv", bufs=3))
            g1_ps = g1s.enter_context(tc.tile_pool(name="g1_ps", bufs=4,
                                                   space="PSUM"))
            g1_out = g1s.enter_context(tc.tile_pool(name="g1_out", bufs=4))
            for pi, (wsrc, dst, with_bias) in enumerate(
                    [(wih1T_f, gx1f, True), (wih1T_b, gx1b, False)]):
                w_sb = w1p.tile([128, KH, 3 * H], bf16, tag="w1")
                nc.sync.dma_start(w_sb[:],
                                  wsrc.rearrange("(k p) m -> p k m", p=128))
                for c in range(NTB):
                    mv = g1_mv.tile([128, TB, KH * BA], bf16, tag="g1mv")
                    nc.sync.dma_start(mv[:],
                                      agbuf[pi, :, c * TB:(c + 1) * TB, :])
                    for m in range(MCH):
                        ps = g1_ps.tile([128, TB * BA], f32, tag="ps")
                        for k in range(KH):
                            nc.tensor.matmul(
                                ps[:], w_sb[:, k, 128 * m:128 * (m + 1)],
                                mv[:, :, BA * k:BA * (k + 1)],
                                start=(k == 0),
                                stop=(not with_bias and k == KH - 1))
                        if with_bias:
                            nc.tensor.matmul(
                                ps[:], bias1_sb[:, 128 * m:128 * (m + 1)],
                                ones_big.bitcast(f32r), start=False, stop=True)
                        ot = g1_out.tile([128, TB, BA], f32, tag="ot")
                        flat = ot[:].rearrange("p t b -> p (t b)")
                        if m % 2 == 0:
                            nc.vector.tensor_copy(flat, ps[:])
                        else:
                            nc.scalar.copy(flat, ps[:])
                        nc.scalar.dma_start(
                            dst[:, m, c * TB:(c + 1) * TB, :], ot[:])

        # ================ layer-1 backward: single step at t = T-1 =========
        with contextlib.ExitStack() as lbs:
            lb_wp = lbs.enter_context(tc.tile_pool(name="lb_wp", bufs=1))
            l1b_w = lb_wp.tile([128, 2 * KH, 3 * H], bf16, tag="l1b_w")
            nc.sync.dma_start(l1b_w[:],
                              wih1bT.rearrange("(k p) m -> p k m", p=128))
            for j in range(KH):
                nc.sync.dma_start(x1last[:, j, :],
                                  agbuf[0, :, T - 1, BA * j:BA * (j + 1)])
                nc.sync.dma_start(x1last[:, KH + j, :],
                                  agbuf[1, :, 0, BA * j:BA * (j + 1)])
            l1b_ps_p = lbs.enter_context(
                tc.tile_pool(name="l1b_ps", bufs=1, space="PSUM"))
            l1b_ps = l1b_ps_p.tile([128, MCH * BA], f32, tag="l1b_ps")
            for m in range(MCH):
                for k in range(2 * KH):
                    nc.tensor.matmul(l1b_ps[:, BA * m:BA * (m + 1)],
                                     l1b_w[:, k, 128 * m:128 * (m + 1)],
                                     x1last[:, k, :],
                                     start=(k == 0), stop=(k == 2 * KH - 1))
            for m in range(MCH):
                nc.vector.tensor_scalar_add(gxl[:, BA * m:BA * (m + 1)],
                                            l1b_ps[:, BA * m:BA * (m + 1)],
                                            b1b_sb[:, m:m + 1])
            nc.scalar.activation(rl[:], gxl[:, 0:4 * BA], SIG)
            nc.scalar.activation(zpl[:], gxl[:, 4 * BA:8 * BA], SIG, scale=-1.0)
            for jj in range(KH):
                nc.vector.scalar_tensor_tensor(
                    n1l[:, BA * jj:BA * (jj + 1)], rl[:, BA * jj:BA * (jj + 1)],
                    bhn1b_sb[:, jj:jj + 1],
                    gxl[:, 8 * BA + BA * jj:8 * BA + BA * (jj + 1)],
                    ALU.mult, ALU.add)
            nc.scalar.activation(ntl[:], n1l[:], TANH)
            nc.vector.tensor_mul(h1bk[:], zpl[:], ntl[:])

        # ================ Phase B: layer-1 forward scan ================
        def gx1_load(t, pool):
            ga = pool.tile([128, MCH * BA], f32, tag="gxa")
            nc.sync.dma_start(ga[:].rearrange("p (m b) -> p m b", m=MCH),
                              gx1f[:, :, t, :])
            gb = pool.tile([128, MCH * BA], f32, tag="gxb")
            nc.sync.dma_start(gb[:].rearrange("p (m b) -> p m b", m=MCH),
                              gx1b[:, :, T - 1 - t, :])
            gs = pool.tile([128, MCH * BA], f32, tag="gxs")
            nc.vector.tensor_add(gs[:], ga[:], gb[:])
            return gs

        h1f_bf = scan(h1a, h1b_, gx1_load, whh1_sb, bhn1_sb, None, "s1")
        h1f = const_tile([128, KH * BA], f32r, "h1f_r")
        nc.vector.tensor_copy(h1f[:], h1f_bf[:])

        # ================ FC ================
        with contextlib.ExitStack() as fcs:
            fc_ps_p = fcs.enter_context(
                tc.tile_pool(name="fc_ps", bufs=1, space="PSUM"))
            fc_ps_t = fc_ps_p.tile([128, BA], f32, tag="fc_ps", name="fc_ps")
            fc_ps = fc_ps_t[0:C, :]
            for k in range(KH):
                nc.tensor.matmul(fc_ps, fcw_sb[:, k, :],
                                 h1f[:, BA * k:BA * (k + 1)],
                                 start=(k == 0), stop=False)
            for k in range(KH):
                nc.tensor.matmul(fc_ps, fcw_sb[:, KH + k, :],
                                 h1bk[:, BA * k:BA * (k + 1)],
                                 start=False, stop=(k == KH - 1))
            nc.scalar.activation(out_sb, fc_ps, IDENT, bias=fcb_sb)
            nc.sync.dma_start(outT[:], out_sb)

    nc.compile()
    return nc


def _make_in_maps(inputs, T):
    x = np.asarray(inputs["x"], dtype=np.float32)

    import ml_dtypes
    bf = ml_dtypes.bfloat16

    def layer_params(wih, whh, bih, bhh):
        wih, whh = np.asarray(wih), np.asarray(whh)
        bih, bhh = np.asarray(bih), np.asarray(bhh)
        bias = (bih + bhh).astype(np.float32).copy()
        bias[2 * H:] = bih[2 * H:]
        return {
            "wihT": np.ascontiguousarray(wih.T, dtype=np.float32),
            "whhT": np.ascontiguousarray(whh.T).astype(bf),
            "bias": bias.reshape(1, 3 * H),
            "bhn": bhh[2 * H:].reshape(1, H).astype(bf),
        }

    l0f = layer_params(inputs["w_ih_l0f"], inputs["w_hh_l0f"],
                       inputs["b_ih_l0f"], inputs["b_hh_l0f"])
    l0b = layer_params(inputs["w_ih_l0b"], inputs["w_hh_l0b"],
                       inputs["b_ih_l0b"], inputs["b_hh_l0b"])
    l1f = layer_params(inputs["w_ih_l1f"], inputs["w_hh_l1f"],
                       inputs["b_ih_l1f"], inputs["b_hh_l1f"])

    wih1fT = np.ascontiguousarray(np.asarray(inputs["w_ih_l1f"]).T
                                  ).astype(bf)  # [2H, 3H]
    wih1bT = np.ascontiguousarray(np.asarray(inputs["w_ih_l1b"]).T).astype(bf)

    b1b = (np.asarray(inputs["b_ih_l1b"]) + np.asarray(inputs["b_hh_l1b"])
           ).astype(np.float32).copy()
    b1b[2 * H:] = np.asarray(inputs["b_ih_l1b"])[2 * H:]
    bias1b_sc = np.ascontiguousarray(b1b.reshape(MCH, 128).T)
    bhn1b_sc = np.ascontiguousarray(
        np.asarray(inputs["b_hh_l1b"])[2 * H:].reshape(KH, 128).T
        .astype(np.float32))

    fcwT = np.ascontiguousarray(np.asarray(inputs["fc_w"]).T, dtype=np.float32)
    fcb = np.asarray(inputs["fc_b"]).reshape(C, 1).astype(np.float32)

    common = {
        "wih1T_f": np.ascontiguousarray(wih1fT[:H]),
        "wih1T_b": np.ascontiguousarray(wih1fT[H:]),
        "bias1": l1f["bias"],
        "bhn1": l1f["bhn"],
        "whh1T": l1f["whhT"],
        "wih1bT": wih1bT,
        "bias1b_sc": bias1b_sc,
        "bhn1b_sc": bhn1b_sc,
        "fcwT": fcwT,
        "fcb": fcb,
    }

    in_maps = []
    for i in range(NCORES):
        p = i % 4
        back = i >= 4
        xs = x[8 * p:8 * p + 8, :T, :]
        if back:
            xs = xs[:, ::-1, :]
        xTl = np.ascontiguousarray(xs.transpose(2, 1, 0))  # [I, T, BA]
        lp = l0b if back else l0f
        m = {
            "xT": xTl,
            "wihT0": lp["wihT"],
            "bias0": lp["bias"],
            "bhn0": lp["bhn"],
            "whhT0": lp["whhT"],
        }
        m.update(common)
        in_maps.append(m)
    return in_maps


def _run(nc, in_maps, trace=False, trace_kwargs=None):
    from concourse.bass_utils import run_bass_kernel_spmd

    last_err = None
    for _ in range(3):
        try:
            return run_bass_kernel_spmd(nc, in_maps,
                                        core_ids=list(range(NCORES)),
                                        trace=trace,
                                        **(trace_kwargs or {}))
        except Exception as e:  # transient NRT device errors
            last_err = e
            import time
            time.sleep(5)
    raise last_err


def kernel(**inputs):
    T = np.asarray(inputs["x"]).shape[1]
    if T not in _PROGRAM_CACHE:
        _PROGRAM_CACHE[T] = _build(T)
    nc = _PROGRAM_CACHE[T]
    in_maps = _make_in_maps(inputs, T)
    res = _run(nc, in_maps)
    out = np.zeros((B, C), dtype=np.float32)
    for p in range(4):
        out[8 * p:8 * p + 8, :] = res.results[p]["outT"].T
    return out
